# revision 20
# baseline (speedup 1.0000x reference)
"""Trainium2 Bass kernel for nn_MlroleNode_64716567216639 (GAT message passing).

Math note: the reference computes a dense NxN GATv2 attention but only row 0
of the output (gat_out[0]) feeds the final MLP, so this kernel computes just
that row: e[j,h] = leaky(g_l[j] + g_r[0]) . w_attn over the 1024 source nodes,
softmax, weighted sum of g_r, then the 3-layer type-define MLP over the 1023
ambiguous nodes.  All 8 cores replicate the attention row; the MLP is sharded
128 nodes per core (per-core `mlpamb` input).

Schedule: inputs stream over all three DMA-capable queues (sync, scalar,
gpsimd); the serial h1 role-routing prologue's weights ride in the first DMA
(`pro`), and the prologue chain runs its leaky on DVE so the scalar/ACT
engine is free to issue DMAs.  tile_wait_until floors keep the DMA-gated
attention matmuls out of the PE queue until the prologue chain has drained
(the scheduler's DMA model is optimistic; an early-slotted matmul would
stall the in-order PE queue).  leaky(gl + g_r[0]) fuses the per-partition
bias: ACT Prelu for three quarters, DVE tensor_scalar+STT for the fourth,
balancing ACT and DVE.  exp runs on ACT with accumulated row sums; the
exp-weighted value reduction is one DVE STT per block with accum_out.  The
MLP uses single-copy weights with Prelu (bias via the ACT bias operand) and
ends in tanh; sigmoid's affine 0.5+0.5*x is applied on host.
"""
import numpy as np

H = 64
N_AMB = 1023
N = 1024            # 1023 ambiguous + node 0 (in column 1023)
HEADS = 4
HID = 64
RT = 4
APT = 3
SLOPE = 0.2
NCORES = 8
SHARD = 128

# proa [65, 141]: WselfT_aug | WmLT | hid_aug | ta ; prob [65, 256]: 4x WC_t_aug
PA_WSELF = 0
PA_WML = 64
PA_HID = 128
PA_TA = 129
PA_COLS = 141
PB_COLS = 256
# wlr [64, 512]: W_l.T | W_r.T
WLR_WL = 0
WLR_WR = 256
# mlpw [128, 260]: G | Wd0aT | Wd1T | Wd2T
MW_G = 0
MW_WD0A = 64
MW_WD1 = 128
MW_WD2 = 256
MW_COLS = 260

# tile_wait_until floors (ms of sim time) for DMA-gated matmul groups
W_GLP = [0.003, 0.0034, 0.0044, 0.005]   # per gl-piece floors (interleave chain)
W_HC = [0.0046, 0.0052]
W_GRP = [0.0054, 0.008]     # per-block gr floors
W_EP = [0.0058, 0.0075]      # per-block e floors
W_Y0 = 0.009
W_WARM = 0.0035

_compiled = None


def _build(do_compile=True):
    import concourse.tile as tile
    from concourse import bacc, mybir

    f32 = mybir.dt.float32
    bf16 = mybir.dt.bfloat16
    AF = mybir.ActivationFunctionType
    ALU = mybir.AluOpType
    AX = mybir.AxisListType

    nc = bacc.Bacc("TRN2", target_bir_lowering=False, debug=False,
                   enable_asserts=False, num_devices=NCORES)

    proa_d = nc.dram_tensor("proa", [65, PA_COLS], bf16, kind="ExternalInput").ap()
    prob_d = nc.dram_tensor("prob", [65, PB_COLS], bf16, kind="ExternalInput").ap()
    ambe_d = nc.dram_tensor("ambe", [64, 512], bf16, kind="ExternalInput").ap()
    ambl_d = nc.dram_tensor("ambl", [64, 512], bf16, kind="ExternalInput").ap()
    wlr_d = nc.dram_tensor("wlr", [64, 512], bf16, kind="ExternalInput").ap()
    wexp_d = nc.dram_tensor("wexp", [128, 128], bf16, kind="ExternalInput").ap()
    mlpw_d = nc.dram_tensor("mlpw", [128, MW_COLS], bf16, kind="ExternalInput").ap()
    mlpamb_d = nc.dram_tensor("mlpamb", [64, SHARD], bf16, kind="ExternalInput").ap()
    biasc_d = nc.dram_tensor("biasc", [128, 3], f32, kind="ExternalInput").ap()
    outT_d = nc.dram_tensor("outT", [RT, SHARD], f32, kind="ExternalOutput").ap()

    PAD_BIG = [128, 1024]   # 2 PSUM banks

    with tile.TileContext(nc) as tc:
        with tc.tile_pool(name="wp", bufs=1) as wp, \
             tc.tile_pool(name="sb", bufs=1) as sb, \
             tc.tile_pool(name="ps", bufs=1, space="PSUM") as ps:

            # ---- input DMAs, critical-first, three queues ----
            pro = wp.tile([65, PA_COLS], bf16, tag="proa")
            nc.sync.dma_start(pro[:], proa_d[:])
            ambl = wp.tile([64, 512], bf16, tag="ambl")
            nc.sync.dma_start(ambl[:], ambl_d[:])
            wexp = wp.tile([128, 128], bf16, tag="wexp")
            nc.sync.dma_start(wexp[:], wexp_d[:])
            biasc = wp.tile([128, 3], f32, tag="biasc")
            nc.sync.dma_start(biasc[:], biasc_d[:])

            prob = wp.tile([65, PB_COLS], bf16, tag="prob")
            nc.scalar.dma_start(prob[:], prob_d[:])
            wlr = wp.tile([64, 512], bf16, tag="wlr")
            nc.scalar.dma_start(wlr[:], wlr_d[:])

            ambe = wp.tile([64, 512], bf16, tag="ambe")
            nc.gpsimd.dma_start(ambe[:], ambe_d[:])
            mlpw = wp.tile([128, MW_COLS], bf16, tag="mlpw")
            nc.gpsimd.dma_start(mlpw[:], mlpw_d[:])
            mlpamb = wp.tile([64, SHARD], bf16, tag="mlpamb")
            nc.gpsimd.dma_start(mlpamb[:], mlpamb_d[:])

            # ---- ACT table warm (Exp/Prelu/Tanh share one set); floored so
            #      the auto-inserted table load lands after the DMA issues ----
            warm = sb.tile([1, 1], f32, tag="warm")
            nc.vector.memset(warm[:], 0.0)
            warm_o = sb.tile([1, 1], f32, tag="warmo")
            with tc.tile_wait_until(W_WARM, enable=W_WARM > 0):
                nc.scalar.activation(warm_o[:], warm[:], AF.Exp)

            def leaky_dve(out_ap, in_ap):
                nc.vector.scalar_tensor_tensor(out=out_ap, in0=in_ap, scalar=SLOPE,
                                               in1=in_ap, op0=ALU.mult, op1=ALU.max)

            # ---- prologue: role-routing chain for node 0 ----
            tsum = sb.tile([65, RT], bf16, tag="tsum")
            nc.vector.memset(tsum[64:65, :], 1.0)
            with nc.allow_low_precision(reason="3-way sum of bf16 agent vectors"):
                nc.vector.reduce_sum(
                    tsum[0:64, :],
                    pro[0:64, PA_TA:PA_TA + RT * APT].rearrange("p (t a) -> p t a",
                                                                a=APT),
                    axis=AX.X)
            h1_ps = ps.tile([H, 1], f32, tag="B", bufs=2, padded_shape=PAD_BIG,
                            name="h1_ps")
            nc.tensor.matmul(h1_ps[:], pro[0:65, PA_WSELF:PA_WSELF + H],
                             pro[0:65, PA_HID:PA_HID + 1], start=True, stop=True)
            h1 = sb.tile([H, 1], bf16, tag="h1", bufs=2)
            nc.vector.tensor_copy(h1[:], h1_ps[:])

            C_ps = ps.tile([H, RT], f32, tag="B", bufs=2, padded_shape=PAD_BIG,
                           name="C_ps")
            for t in range(RT):
                nc.tensor.matmul(C_ps[:, t:t + 1],
                                 prob[0:65, H * t:H * (t + 1)],
                                 tsum[:, t:t + 1], start=True, stop=True)
            C_sb = sb.tile([H, RT], f32, tag="Csb")
            nc.vector.tensor_copy(C_sb[:, 0:1], C_ps[:, 0:1])
            nc.vector.tensor_copy(C_sb[:, 1:RT], C_ps[:, 1:RT])

            for t in range(RT):
                u_ps = ps.tile([H, 1], f32, tag="B", bufs=2, padded_shape=PAD_BIG,
                               name=f"u_ps{t}")
                nc.tensor.matmul(u_ps[:], pro[0:64, PA_WML:PA_WML + H], h1[:],
                                 start=True, stop=True)
                uu = sb.tile([H, 1], bf16, tag="uu", bufs=2)
                nc.vector.tensor_scalar_add(uu[:], u_ps[:], C_sb[:, t:t + 1])
                h1n = sb.tile([H, 1], bf16, tag="h1", bufs=2)
                leaky_dve(h1n[:], uu[:])
                h1 = h1n

            # g_r[0] = W_r @ h1 -> per-partition bias columns for both blocks
            gq_ps = ps.tile([128, 2], f32, tag="B", bufs=2, padded_shape=PAD_BIG,
                            name="gq_ps")
            for b in range(2):
                nc.tensor.matmul(gq_ps[:, b:b + 1],
                                 wlr[0:64, WLR_WR + 128 * b:WLR_WR + 128 * (b + 1)],
                                 h1[:], start=True, stop=True)
            gr0c = sb.tile([128, 2], f32, tag="gr0c")
            nc.vector.tensor_copy(gr0c[:], gq_ps[:])

            # ---- g_l for all 1024 nodes (cols 0:512 from ambe, 512:1023 from
            #      ambl, col 1023 = h1) ----
            gl_b = [ps.tile([128, N], f32, tag="A", bufs=2, name=f"gl{b}")
                    for b in range(2)]
            gl_pieces = [(0, 0, 512, "ambe"), (1, 0, 512, "ambe"),
                         (0, 512, N_AMB, "ambl"), (1, 512, N_AMB, "ambl")]
            for (b, lo, hi, src_name), wfl in zip(gl_pieces, W_GLP):
                wl = wlr[0:64, WLR_WL + 128 * b:WLR_WL + 128 * (b + 1)]
                rhs = ambe[:] if src_name == "ambe" else ambl[:, 0:511]
                with tc.tile_wait_until(wfl, enable=wfl > 0):
                    nc.tensor.matmul(gl_b[b][:, lo:hi], wl, rhs, start=True,
                                     stop=True)
            for b, whc in ((0, W_HC[0]), (1, W_HC[1])):
                with tc.tile_wait_until(whc, enable=whc > 0):
                    wl = wlr[0:64, WLR_WL + 128 * b:WLR_WL + 128 * (b + 1)]
                    nc.tensor.matmul(gl_b[b][:, N_AMB:N], wl, h1[:], start=True,
                                     stop=True)

            # ---- g_r values for all 1024 nodes ----
            gr_b = [ps.tile([128, N], f32, tag="B", bufs=2, name=f"gr{b}")
                    for b in range(2)]
            for b in range(2):
                with tc.tile_wait_until(W_GRP[b], enable=W_GRP[b] > 0):
                    wr = wlr[0:64, WLR_WR + 128 * b:WLR_WR + 128 * (b + 1)]
                    nc.tensor.matmul(gr_b[b][:, 0:512], wr, ambe[:], start=True,
                                     stop=True)
                    nc.tensor.matmul(gr_b[b][:, 512:N_AMB], wr, ambl[:, 0:511],
                                     start=True, stop=True)
                    nc.tensor.matmul(gr_b[b][:, N_AMB:N], wr, h1[:], start=True,
                                     stop=True)

            # ---- leaky(gl + gr0): 3 quarters on ACT (Prelu, fused bias),
            #      block1 cols 512:1024 on DVE (2-pass) ----
            tsb0 = sb.tile([128, N], bf16, tag="tsb0")
            nc.scalar.activation(tsb0[:, 0:512], gl_b[0][:, 0:512], AF.Prelu,
                                 bias=gr0c[:, 0:1], alpha=SLOPE)
            nc.scalar.activation(tsb0[:, 512:N], gl_b[0][:, 512:N], AF.Prelu,
                                 bias=gr0c[:, 0:1], alpha=SLOPE)
            tsb1 = sb.tile([128, N], bf16, tag="tsb1")
            u1b = sb.tile([128, 512], bf16, tag="u1b")
            nc.vector.tensor_scalar_add(u1b[:], gl_b[1][:, 512:N], gr0c[:, 1:2])
            leaky_dve(tsb1[:, 512:N], u1b[:])
            nc.scalar.activation(tsb1[:, 0:512], gl_b[1][:, 0:512], AF.Prelu,
                                 bias=gr0c[:, 1:2], alpha=SLOPE)

            # ---- e = wexp.T @ leaky, exp with accumulated denominators ----
            tsbs = (tsb0, tsb1)
            ssum = sb.tile([128, 2], f32, tag="ssum")
            att_u = sb.tile([128, 2], f32, tag="attu")
            scrx = sb.tile([128, N], bf16, tag="scrx")
            pexp = [None, None]
            e_b = [None, None]
            for b in range(2):
                with tc.tile_wait_until(W_EP[b], enable=W_EP[b] > 0):
                    e_b[b] = ps.tile([128, N], f32, tag="A", bufs=2, name=f"e{b}")
                    for lo, hi in ((0, 512), (512, N)):
                        nc.tensor.matmul(e_b[b][:, lo:hi], wexp[:],
                                         tsbs[b][:, lo:hi], start=True, stop=True)
            for b in range(2):
                pexp[b] = sb.tile([128, N], bf16, tag="pex", bufs=2,
                                  name=f"pexp{b}")
                nc.scalar.activation(pexp[b][:], e_b[b][:], AF.Exp,
                                     accum_out=ssum[:, b:b + 1])

            # ---- exp-weighted value sums (fused mul + row-accumulate) ----
            rs = sb.tile([128, 2], f32, tag="rs")
            nc.vector.scalar_tensor_tensor(
                out=scrx[:], in0=pexp[0][:], scalar=1.0, in1=gr_b[0][:],
                op0=ALU.mult, op1=ALU.mult, accum_out=att_u[:, 0:1])
            nc.vector.reciprocal(rs[:], ssum[:])
            nc.vector.scalar_tensor_tensor(
                out=scrx[:], in0=pexp[1][:], scalar=1.0, in1=gr_b[1][:],
                op0=ALU.mult, op1=ALU.mult, accum_out=att_u[:, 1:2])

            # ---- softmax normalize + head mean prep ----
            att_n = sb.tile([128, 2], bf16, tag="attn")
            nc.vector.tensor_tensor(att_n[:], att_u[:], rs[:], op=ALU.mult)
            att_s = sb.tile([128, 1], bf16, tag="atts")
            nc.vector.tensor_tensor(att_s[:], att_n[:, 0:1], att_n[:, 1:2],
                                    op=ALU.add)

            # ---- final MLP on this core's 128-node shard ----
            y0_ps = ps.tile([H, SHARD], f32, tag="A", bufs=2,
                            padded_shape=PAD_BIG, name="y0_ps")
            with tc.tile_wait_until(W_Y0, enable=W_Y0 > 0):
                nc.tensor.matmul(y0_ps[:], mlpw[0:64, MW_WD0A:MW_WD0A + H],
                                 mlpamb[:], start=True, stop=False)
            nc.tensor.matmul(y0_ps[:], mlpw[0:128, MW_G:MW_G + H],
                             att_s[:, 0:1].broadcast_to([128, SHARD]),
                             start=False, stop=True)
            x1 = sb.tile([H, SHARD], bf16, tag="x1")
            nc.scalar.activation(x1[:], y0_ps[:], AF.Prelu,
                                 bias=biasc[0:64, 0:1], alpha=SLOPE)
            y1_ps = ps.tile([128, SHARD], f32, tag="A", bufs=2,
                            padded_shape=PAD_BIG, name="y1_ps")
            nc.tensor.matmul(y1_ps[:], mlpw[0:64, MW_WD1:MW_WD1 + 128], x1[:],
                             start=True, stop=True)
            x2 = sb.tile([128, SHARD], bf16, tag="x2")
            nc.scalar.activation(x2[:], y1_ps[:], AF.Prelu,
                                 bias=biasc[:, 1:2], alpha=SLOPE)
            o_ps = ps.tile([RT, SHARD], f32, tag="A", bufs=2,
                           padded_shape=PAD_BIG, name="o_ps")
            nc.tensor.matmul(o_ps[:], mlpw[0:128, MW_WD2:MW_WD2 + RT], x2[:],
                             start=True, stop=True)
            # sigmoid(z) = 0.5 + 0.5*tanh(0.5*z + 0.5*bd2); affine on host
            th = sb.tile([RT, SHARD], f32, tag="th")
            nc.scalar.activation(th[:], o_ps[:], AF.Tanh, bias=biasc[0:4, 2:3],
                                 scale=0.5)
            nc.sync.dma_start(outT_d[:], th[:])

    if do_compile:
        nc.compile()
    return nc


def _prep_inputs(inputs):
    import ml_dtypes
    f32 = np.float32
    bf16 = ml_dtypes.bfloat16

    def bf(a):
        return np.ascontiguousarray(np.asarray(a, f32)).astype(bf16)

    hidden = np.asarray(inputs["hidden"], f32)
    ambiguous = np.asarray(inputs["ambiguous"], f32)
    type_agents = np.asarray(inputs["type_agents"], f32)
    W_self = np.asarray(inputs["W_self"], f32)
    b_self = np.asarray(inputs["b_self"], f32)
    W_merge = np.asarray(inputs["W_merge"], f32)
    b_merge = np.asarray(inputs["b_merge"], f32)
    W_trans = np.asarray(inputs["W_trans"], f32)
    b_trans = np.asarray(inputs["b_trans"], f32)
    W_l = np.asarray(inputs["W_l"], f32)
    W_r = np.asarray(inputs["W_r"], f32)
    w_attn = np.asarray(inputs["w_attn"], f32)
    Wd0 = np.asarray(inputs["Wd0"], f32)
    bd0 = np.asarray(inputs["bd0"], f32)
    Wd1 = np.asarray(inputs["Wd1"], f32)
    bd1 = np.asarray(inputs["bd1"], f32)
    Wd2 = np.asarray(inputs["Wd2"], f32)
    bd2 = np.asarray(inputs["bd2"], f32)

    WmR = W_merge[:, H:]

    proa = np.zeros((65, PA_COLS), f32)
    proa[0:64, PA_WSELF:PA_WSELF + H] = W_self.T
    proa[64, PA_WSELF:PA_WSELF + H] = b_self
    proa[0:64, PA_WML:PA_WML + H] = W_merge[:, :H].T
    proa[0:64, PA_HID] = hidden[0]
    proa[64, PA_HID] = 1.0
    proa[0:64, PA_TA:PA_TA + RT * APT] = type_agents.reshape(RT * APT, H).T
    prob = np.zeros((65, PB_COLS), f32)
    for t in range(RT):
        WC = WmR @ W_trans[t] / APT
        bC = WmR @ b_trans[t] + b_merge
        prob[0:64, H * t:H * (t + 1)] = WC.T
        prob[64, H * t:H * (t + 1)] = bC

    ambT = ambiguous.T  # [64, 1023]
    ambe = ambT[:, 0:512]
    ambl = np.zeros((64, 512), f32)
    ambl[:, 0:511] = ambT[:, 512:1023]

    wlr = np.zeros((64, 512), f32)
    wlr[:, WLR_WL:WLR_WL + 256] = W_l.T
    wlr[:, WLR_WR:WLR_WR + 256] = W_r.T

    wexp = np.zeros((128, 128), f32)
    for hh in range(2):
        wexp[hh * 64:(hh + 1) * 64, hh * 64:(hh + 1) * 64] = w_attn[:, None]

    mlpw = np.zeros((128, MW_COLS), f32)
    G = 0.25 * Wd0[:, H:].T  # fold mean-over-4-heads into Wd0b
    mlpw[0:64, MW_G:MW_G + H] = G
    mlpw[64:128, MW_G:MW_G + H] = G
    mlpw[0:64, MW_WD0A:MW_WD0A + H] = Wd0[:, :H].T
    mlpw[0:64, MW_WD1:MW_WD1 + 128] = Wd1.T
    mlpw[0:128, MW_WD2:MW_WD2 + RT] = Wd2.T

    biasc = np.zeros((128, 3), f32)
    biasc[0:64, 0] = bd0
    biasc[:, 1] = bd1
    biasc[0:4, 2] = 0.5 * bd2

    shared = {
        "proa": bf(proa),
        "prob": bf(prob),
        "ambe": bf(ambe),
        "ambl": bf(ambl),
        "wlr": bf(wlr),
        "wexp": bf(wexp),
        "mlpw": bf(mlpw),
        "biasc": np.ascontiguousarray(biasc),
    }
    amb_pad = np.zeros((64, NCORES * SHARD), f32)
    amb_pad[:, :N_AMB] = ambT
    in_maps = []
    for cidx in range(NCORES):
        m = dict(shared)
        m["mlpamb"] = bf(amb_pad[:, cidx * SHARD:(cidx + 1) * SHARD])
        in_maps.append(m)
    return in_maps


def kernel(**inputs) -> np.ndarray:
    global _compiled
    if _compiled is None:
        _compiled = _build()
    nc = _compiled
    from concourse import bass_utils

    in_maps = _prep_inputs(inputs)
    res = bass_utils.run_bass_kernel_spmd(nc, in_maps, core_ids=list(range(NCORES)))
    out = np.empty((N_AMB, RT), np.float32)
    for cidx in range(NCORES):
        lo = cidx * SHARD
        hi = min(lo + SHARD, N_AMB)
        # device returns tanh(z/2 + bd2/2); sigmoid(z+bd2) = 0.5 + 0.5*that
        out[lo:hi, :] = 0.5 + 0.5 * res.results[cidx]["outT"][:, :hi - lo].T
    return out


# revision 21
# speedup vs baseline: 1.0115x; 1.0115x over previous
"""Trainium2 Bass kernel for nn_MlroleNode_64716567216639 (GAT message passing).

Math note: the reference computes a dense NxN GATv2 attention but only row 0
of the output (gat_out[0]) feeds the final MLP, so this kernel computes just
that row: e[j,h] = leaky(g_l[j] + g_r[0]) . w_attn over the 1024 source nodes,
softmax, weighted sum of g_r, then the 3-layer type-define MLP over the 1023
ambiguous nodes.  All 8 cores replicate the attention row; the MLP is sharded
128 nodes per core (per-core `mlpamb` input).

Schedule: inputs stream over all three DMA-capable queues (sync, scalar,
gpsimd); the serial h1 role-routing prologue's weights ride in the first DMA
(`pro`), and the prologue chain runs its leaky on DVE so the scalar/ACT
engine is free to issue DMAs.  tile_wait_until floors keep the DMA-gated
attention matmuls out of the PE queue until the prologue chain has drained
(the scheduler's DMA model is optimistic; an early-slotted matmul would
stall the in-order PE queue).  leaky(gl + g_r[0]) fuses the per-partition
bias: ACT Prelu for three quarters, DVE tensor_scalar+STT for the fourth,
balancing ACT and DVE.  exp runs on ACT with accumulated row sums; the
exp-weighted value reduction is one DVE STT per block with accum_out.  The
MLP uses single-copy weights with Prelu (bias via the ACT bias operand) and
ends in tanh; sigmoid's affine 0.5+0.5*x is applied on host.
"""
import numpy as np

H = 64
N_AMB = 1023
N = 1024            # 1023 ambiguous + node 0 (in column 1023)
HEADS = 4
HID = 64
RT = 4
APT = 3
SLOPE = 0.2
NCORES = 8
SHARD = 128

# proa [65, 141]: WselfT_aug | WmLT | hid_aug | ta ; prob [65, 256]: 4x WC_t_aug
PA_WSELF = 0
PA_WML = 64
PA_HID = 128
PA_TA = 129
PA_COLS = 141
PB_COLS = 256
# wlr [64, 512]: W_l.T | W_r.T
WLR_WL = 0
WLR_WR = 256
# mlpw [128, 260]: G | Wd0aT | Wd1T | Wd2T
MW_G = 0
MW_WD0A = 64
MW_WD1 = 128
MW_WD2 = 256
MW_COLS = 260

# tile_wait_until floors (ms of sim time) for DMA-gated matmul groups
W_GLP = [0.0034, 0.0038, 0.0044, 0.005]   # per gl-piece floors (interleave chain)
W_HC = [0.0046, 0.0052]
W_GRP = [0.0054, 0.008]     # per-block gr floors
W_EP = [0.0058, 0.0075]      # per-block e floors
W_Y0 = 0.009
W_WARM = 0.0035

_compiled = None


def _build(do_compile=True):
    import concourse.tile as tile
    from concourse import bacc, mybir

    f32 = mybir.dt.float32
    bf16 = mybir.dt.bfloat16
    AF = mybir.ActivationFunctionType
    ALU = mybir.AluOpType
    AX = mybir.AxisListType

    nc = bacc.Bacc("TRN2", target_bir_lowering=False, debug=False,
                   enable_asserts=False, num_devices=NCORES)

    proa_d = nc.dram_tensor("proa", [65, PA_COLS], bf16, kind="ExternalInput").ap()
    prob_d = nc.dram_tensor("prob", [65, PB_COLS], bf16, kind="ExternalInput").ap()
    ambe_d = nc.dram_tensor("ambe", [64, 512], bf16, kind="ExternalInput").ap()
    ambl_d = nc.dram_tensor("ambl", [64, 512], bf16, kind="ExternalInput").ap()
    wlr_d = nc.dram_tensor("wlr", [64, 512], bf16, kind="ExternalInput").ap()
    wexp_d = nc.dram_tensor("wexp", [128, 128], bf16, kind="ExternalInput").ap()
    mlpw_d = nc.dram_tensor("mlpw", [128, MW_COLS], bf16, kind="ExternalInput").ap()
    mlpamb_d = nc.dram_tensor("mlpamb", [64, SHARD], bf16, kind="ExternalInput").ap()
    biasc_d = nc.dram_tensor("biasc", [128, 3], f32, kind="ExternalInput").ap()
    outT_d = nc.dram_tensor("outT", [RT, SHARD], f32, kind="ExternalOutput").ap()

    PAD_BIG = [128, 1024]   # 2 PSUM banks

    with tile.TileContext(nc) as tc:
        with tc.tile_pool(name="wp", bufs=1) as wp, \
             tc.tile_pool(name="sb", bufs=1) as sb, \
             tc.tile_pool(name="ps", bufs=1, space="PSUM") as ps:

            # ---- input DMAs, critical-first, three queues ----
            pro = wp.tile([65, PA_COLS], bf16, tag="proa")
            nc.sync.dma_start(pro[:], proa_d[:])
            ambl = wp.tile([64, 512], bf16, tag="ambl")
            nc.sync.dma_start(ambl[:], ambl_d[:])
            wexp = wp.tile([128, 128], bf16, tag="wexp")
            nc.sync.dma_start(wexp[:], wexp_d[:])
            biasc = wp.tile([128, 3], f32, tag="biasc")
            nc.sync.dma_start(biasc[:], biasc_d[:])

            prob = wp.tile([65, PB_COLS], bf16, tag="prob")
            nc.scalar.dma_start(prob[:], prob_d[:])
            wlr = wp.tile([64, 512], bf16, tag="wlr")
            nc.scalar.dma_start(wlr[:], wlr_d[:])

            ambe = wp.tile([64, 512], bf16, tag="ambe")
            nc.gpsimd.dma_start(ambe[:], ambe_d[:])
            mlpw = wp.tile([128, MW_COLS], bf16, tag="mlpw")
            nc.gpsimd.dma_start(mlpw[:], mlpw_d[:])
            mlpamb = wp.tile([64, SHARD], bf16, tag="mlpamb")
            nc.gpsimd.dma_start(mlpamb[:], mlpamb_d[:])

            # ---- ACT table warm (Exp/Prelu/Tanh share one set); floored so
            #      the auto-inserted table load lands after the DMA issues ----
            warm = sb.tile([1, 1], f32, tag="warm")
            nc.vector.memset(warm[:], 0.0)
            warm_o = sb.tile([1, 1], f32, tag="warmo")
            with tc.tile_wait_until(W_WARM, enable=W_WARM > 0):
                nc.scalar.activation(warm_o[:], warm[:], AF.Exp)

            def leaky_dve(out_ap, in_ap):
                nc.vector.scalar_tensor_tensor(out=out_ap, in0=in_ap, scalar=SLOPE,
                                               in1=in_ap, op0=ALU.mult, op1=ALU.max)

            # ---- prologue: role-routing chain for node 0 ----
            tsum = sb.tile([65, RT], bf16, tag="tsum")
            nc.vector.memset(tsum[64:65, :], 1.0)
            with nc.allow_low_precision(reason="3-way sum of bf16 agent vectors"):
                nc.vector.reduce_sum(
                    tsum[0:64, :],
                    pro[0:64, PA_TA:PA_TA + RT * APT].rearrange("p (t a) -> p t a",
                                                                a=APT),
                    axis=AX.X)
            h1_ps = ps.tile([H, 1], f32, tag="B", bufs=2, padded_shape=PAD_BIG,
                            name="h1_ps")
            nc.tensor.matmul(h1_ps[:], pro[0:65, PA_WSELF:PA_WSELF + H],
                             pro[0:65, PA_HID:PA_HID + 1], start=True, stop=True)
            h1 = sb.tile([H, 1], bf16, tag="h1", bufs=2)
            nc.vector.tensor_copy(h1[:], h1_ps[:])

            C_ps = ps.tile([H, RT], f32, tag="B", bufs=2, padded_shape=PAD_BIG,
                           name="C_ps")
            for t in range(RT):
                nc.tensor.matmul(C_ps[:, t:t + 1],
                                 prob[0:65, H * t:H * (t + 1)],
                                 tsum[:, t:t + 1], start=True, stop=True)
            C_sb = sb.tile([H, RT], f32, tag="Csb")
            nc.vector.tensor_copy(C_sb[:, 0:1], C_ps[:, 0:1])
            nc.vector.tensor_copy(C_sb[:, 1:RT], C_ps[:, 1:RT])

            for t in range(RT):
                u_ps = ps.tile([H, 1], f32, tag="B", bufs=2, padded_shape=PAD_BIG,
                               name=f"u_ps{t}")
                nc.tensor.matmul(u_ps[:], pro[0:64, PA_WML:PA_WML + H], h1[:],
                                 start=True, stop=True)
                uu = sb.tile([H, 1], bf16, tag="uu", bufs=2)
                nc.vector.tensor_scalar_add(uu[:], u_ps[:], C_sb[:, t:t + 1])
                h1n = sb.tile([H, 1], bf16, tag="h1", bufs=2)
                leaky_dve(h1n[:], uu[:])
                h1 = h1n

            # g_r[0] = W_r @ h1 -> per-partition bias columns for both blocks
            gq_ps = ps.tile([128, 2], f32, tag="B", bufs=2, padded_shape=PAD_BIG,
                            name="gq_ps")
            for b in range(2):
                nc.tensor.matmul(gq_ps[:, b:b + 1],
                                 wlr[0:64, WLR_WR + 128 * b:WLR_WR + 128 * (b + 1)],
                                 h1[:], start=True, stop=True)
            gr0c = sb.tile([128, 2], f32, tag="gr0c")
            nc.vector.tensor_copy(gr0c[:], gq_ps[:])

            # ---- g_l for all 1024 nodes (cols 0:512 from ambe, 512:1023 from
            #      ambl, col 1023 = h1) ----
            gl_b = [ps.tile([128, N], f32, tag="A", bufs=2, name=f"gl{b}")
                    for b in range(2)]
            gl_pieces = [(0, 0, 512, "ambe"), (1, 0, 512, "ambe"),
                         (0, 512, N_AMB, "ambl"), (1, 512, N_AMB, "ambl")]
            for (b, lo, hi, src_name), wfl in zip(gl_pieces, W_GLP):
                wl = wlr[0:64, WLR_WL + 128 * b:WLR_WL + 128 * (b + 1)]
                rhs = ambe[:] if src_name == "ambe" else ambl[:, 0:511]
                with tc.tile_wait_until(wfl, enable=wfl > 0):
                    nc.tensor.matmul(gl_b[b][:, lo:hi], wl, rhs, start=True,
                                     stop=True)
            for b, whc in ((0, W_HC[0]), (1, W_HC[1])):
                with tc.tile_wait_until(whc, enable=whc > 0):
                    wl = wlr[0:64, WLR_WL + 128 * b:WLR_WL + 128 * (b + 1)]
                    nc.tensor.matmul(gl_b[b][:, N_AMB:N], wl, h1[:], start=True,
                                     stop=True)

            # ---- g_r values for all 1024 nodes ----
            gr_b = [ps.tile([128, N], f32, tag="B", bufs=2, name=f"gr{b}")
                    for b in range(2)]
            for b in range(2):
                with tc.tile_wait_until(W_GRP[b], enable=W_GRP[b] > 0):
                    wr = wlr[0:64, WLR_WR + 128 * b:WLR_WR + 128 * (b + 1)]
                    nc.tensor.matmul(gr_b[b][:, 0:512], wr, ambe[:], start=True,
                                     stop=True)
                    nc.tensor.matmul(gr_b[b][:, 512:N_AMB], wr, ambl[:, 0:511],
                                     start=True, stop=True)
                    nc.tensor.matmul(gr_b[b][:, N_AMB:N], wr, h1[:], start=True,
                                     stop=True)

            # ---- leaky(gl + gr0): 3 quarters on ACT (Prelu, fused bias),
            #      block1 cols 512:1024 on DVE (2-pass) ----
            tsb0 = sb.tile([128, N], bf16, tag="tsb0")
            nc.scalar.activation(tsb0[:, 0:512], gl_b[0][:, 0:512], AF.Prelu,
                                 bias=gr0c[:, 0:1], alpha=SLOPE)
            nc.scalar.activation(tsb0[:, 512:N], gl_b[0][:, 512:N], AF.Prelu,
                                 bias=gr0c[:, 0:1], alpha=SLOPE)
            tsb1 = sb.tile([128, N], bf16, tag="tsb1")
            u1b = sb.tile([128, 512], bf16, tag="u1b")
            nc.vector.tensor_scalar_add(u1b[:], gl_b[1][:, 512:N], gr0c[:, 1:2])
            leaky_dve(tsb1[:, 512:N], u1b[:])
            nc.scalar.activation(tsb1[:, 0:512], gl_b[1][:, 0:512], AF.Prelu,
                                 bias=gr0c[:, 1:2], alpha=SLOPE)

            # ---- e = wexp.T @ leaky, exp with accumulated denominators ----
            tsbs = (tsb0, tsb1)
            ssum = sb.tile([128, 2], f32, tag="ssum")
            att_u = sb.tile([128, 2], f32, tag="attu")
            scrx = sb.tile([128, N], bf16, tag="scrx")
            pexp = [None, None]
            e_b = [None, None]
            for b in range(2):
                with tc.tile_wait_until(W_EP[b], enable=W_EP[b] > 0):
                    e_b[b] = ps.tile([128, N], f32, tag="A", bufs=2, name=f"e{b}")
                    for lo, hi in ((0, 512), (512, N)):
                        nc.tensor.matmul(e_b[b][:, lo:hi], wexp[:],
                                         tsbs[b][:, lo:hi], start=True, stop=True)
            for b in range(2):
                pexp[b] = sb.tile([128, N], bf16, tag="pex", bufs=2,
                                  name=f"pexp{b}")
                nc.scalar.activation(pexp[b][:], e_b[b][:], AF.Exp,
                                     accum_out=ssum[:, b:b + 1])

            # ---- exp-weighted value sums (fused mul + row-accumulate) ----
            rs = sb.tile([128, 2], f32, tag="rs")
            nc.vector.scalar_tensor_tensor(
                out=scrx[:], in0=pexp[0][:], scalar=1.0, in1=gr_b[0][:],
                op0=ALU.mult, op1=ALU.mult, accum_out=att_u[:, 0:1])
            nc.vector.reciprocal(rs[:], ssum[:])
            nc.vector.scalar_tensor_tensor(
                out=scrx[:], in0=pexp[1][:], scalar=1.0, in1=gr_b[1][:],
                op0=ALU.mult, op1=ALU.mult, accum_out=att_u[:, 1:2])

            # ---- softmax normalize + head mean prep ----
            att_n = sb.tile([128, 2], bf16, tag="attn")
            nc.vector.tensor_tensor(att_n[:], att_u[:], rs[:], op=ALU.mult)
            att_s = sb.tile([128, 1], bf16, tag="atts")
            nc.vector.tensor_tensor(att_s[:], att_n[:, 0:1], att_n[:, 1:2],
                                    op=ALU.add)

            # ---- final MLP on this core's 128-node shard ----
            y0_ps = ps.tile([H, SHARD], f32, tag="A", bufs=2,
                            padded_shape=PAD_BIG, name="y0_ps")
            with tc.tile_wait_until(W_Y0, enable=W_Y0 > 0):
                nc.tensor.matmul(y0_ps[:], mlpw[0:64, MW_WD0A:MW_WD0A + H],
                                 mlpamb[:], start=True, stop=False)
            nc.tensor.matmul(y0_ps[:], mlpw[0:128, MW_G:MW_G + H],
                             att_s[:, 0:1].broadcast_to([128, SHARD]),
                             start=False, stop=True)
            x1 = sb.tile([H, SHARD], bf16, tag="x1")
            nc.scalar.activation(x1[:], y0_ps[:], AF.Prelu,
                                 bias=biasc[0:64, 0:1], alpha=SLOPE)
            y1_ps = ps.tile([128, SHARD], f32, tag="A", bufs=2,
                            padded_shape=PAD_BIG, name="y1_ps")
            nc.tensor.matmul(y1_ps[:], mlpw[0:64, MW_WD1:MW_WD1 + 128], x1[:],
                             start=True, stop=True)
            x2 = sb.tile([128, SHARD], bf16, tag="x2")
            nc.scalar.activation(x2[:], y1_ps[:], AF.Prelu,
                                 bias=biasc[:, 1:2], alpha=SLOPE)
            o_ps = ps.tile([RT, SHARD], f32, tag="A", bufs=2,
                           padded_shape=PAD_BIG, name="o_ps")
            nc.tensor.matmul(o_ps[:], mlpw[0:128, MW_WD2:MW_WD2 + RT], x2[:],
                             start=True, stop=True)
            # sigmoid(z) = 0.5 + 0.5*tanh(0.5*z + 0.5*bd2); affine on host
            th = sb.tile([RT, SHARD], f32, tag="th")
            nc.scalar.activation(th[:], o_ps[:], AF.Tanh, bias=biasc[0:4, 2:3],
                                 scale=0.5)
            nc.sync.dma_start(outT_d[:], th[:])

    if do_compile:
        nc.compile()
    return nc


def _prep_inputs(inputs):
    import ml_dtypes
    f32 = np.float32
    bf16 = ml_dtypes.bfloat16

    def bf(a):
        return np.ascontiguousarray(np.asarray(a, f32)).astype(bf16)

    hidden = np.asarray(inputs["hidden"], f32)
    ambiguous = np.asarray(inputs["ambiguous"], f32)
    type_agents = np.asarray(inputs["type_agents"], f32)
    W_self = np.asarray(inputs["W_self"], f32)
    b_self = np.asarray(inputs["b_self"], f32)
    W_merge = np.asarray(inputs["W_merge"], f32)
    b_merge = np.asarray(inputs["b_merge"], f32)
    W_trans = np.asarray(inputs["W_trans"], f32)
    b_trans = np.asarray(inputs["b_trans"], f32)
    W_l = np.asarray(inputs["W_l"], f32)
    W_r = np.asarray(inputs["W_r"], f32)
    w_attn = np.asarray(inputs["w_attn"], f32)
    Wd0 = np.asarray(inputs["Wd0"], f32)
    bd0 = np.asarray(inputs["bd0"], f32)
    Wd1 = np.asarray(inputs["Wd1"], f32)
    bd1 = np.asarray(inputs["bd1"], f32)
    Wd2 = np.asarray(inputs["Wd2"], f32)
    bd2 = np.asarray(inputs["bd2"], f32)

    WmR = W_merge[:, H:]

    proa = np.zeros((65, PA_COLS), f32)
    proa[0:64, PA_WSELF:PA_WSELF + H] = W_self.T
    proa[64, PA_WSELF:PA_WSELF + H] = b_self
    proa[0:64, PA_WML:PA_WML + H] = W_merge[:, :H].T
    proa[0:64, PA_HID] = hidden[0]
    proa[64, PA_HID] = 1.0
    proa[0:64, PA_TA:PA_TA + RT * APT] = type_agents.reshape(RT * APT, H).T
    prob = np.zeros((65, PB_COLS), f32)
    for t in range(RT):
        WC = WmR @ W_trans[t] / APT
        bC = WmR @ b_trans[t] + b_merge
        prob[0:64, H * t:H * (t + 1)] = WC.T
        prob[64, H * t:H * (t + 1)] = bC

    ambT = ambiguous.T  # [64, 1023]
    ambe = ambT[:, 0:512]
    ambl = np.zeros((64, 512), f32)
    ambl[:, 0:511] = ambT[:, 512:1023]

    wlr = np.zeros((64, 512), f32)
    wlr[:, WLR_WL:WLR_WL + 256] = W_l.T
    wlr[:, WLR_WR:WLR_WR + 256] = W_r.T

    wexp = np.zeros((128, 128), f32)
    for hh in range(2):
        wexp[hh * 64:(hh + 1) * 64, hh * 64:(hh + 1) * 64] = w_attn[:, None]

    mlpw = np.zeros((128, MW_COLS), f32)
    G = 0.25 * Wd0[:, H:].T  # fold mean-over-4-heads into Wd0b
    mlpw[0:64, MW_G:MW_G + H] = G
    mlpw[64:128, MW_G:MW_G + H] = G
    mlpw[0:64, MW_WD0A:MW_WD0A + H] = Wd0[:, :H].T
    mlpw[0:64, MW_WD1:MW_WD1 + 128] = Wd1.T
    mlpw[0:128, MW_WD2:MW_WD2 + RT] = Wd2.T

    biasc = np.zeros((128, 3), f32)
    biasc[0:64, 0] = bd0
    biasc[:, 1] = bd1
    biasc[0:4, 2] = 0.5 * bd2

    shared = {
        "proa": bf(proa),
        "prob": bf(prob),
        "ambe": bf(ambe),
        "ambl": bf(ambl),
        "wlr": bf(wlr),
        "wexp": bf(wexp),
        "mlpw": bf(mlpw),
        "biasc": np.ascontiguousarray(biasc),
    }
    amb_pad = np.zeros((64, NCORES * SHARD), f32)
    amb_pad[:, :N_AMB] = ambT
    in_maps = []
    for cidx in range(NCORES):
        m = dict(shared)
        m["mlpamb"] = bf(amb_pad[:, cidx * SHARD:(cidx + 1) * SHARD])
        in_maps.append(m)
    return in_maps


def kernel(**inputs) -> np.ndarray:
    global _compiled
    if _compiled is None:
        _compiled = _build()
    nc = _compiled
    from concourse import bass_utils

    in_maps = _prep_inputs(inputs)
    res = bass_utils.run_bass_kernel_spmd(nc, in_maps, core_ids=list(range(NCORES)))
    out = np.empty((N_AMB, RT), np.float32)
    for cidx in range(NCORES):
        lo = cidx * SHARD
        hi = min(lo + SHARD, N_AMB)
        # device returns tanh(z/2 + bd2/2); sigmoid(z+bd2) = 0.5 + 0.5*that
        out[lo:hi, :] = 0.5 + 0.5 * res.results[cidx]["outT"][:, :hi - lo].T
    return out
